# revision 20
# baseline (speedup 1.0000x reference)
"""Trainium2 Bass kernel for DGN-style GNN message passing (3x NNConv + pairwise L1 CBT).

Strategy (8 NeuronCores, SPMD, edges sharded by destination node):
 - Core c owns nodes [64c, 64c+64) and all edges targeting them (host sorts
   edges by dst, pads per-core lists to a common chunk count; 128 edges/chunk).
 - Hot tensors are bf16 (PE matmuls are 4x faster than fp32; DVE gets 2x on
   packed bf16 SBUF ops). PSUM stays fp32. Quantization errors on the edge
   path average out over ~512 edges/node; the non-averaging paths (root
   injection, final CBT) run in fp32.
 - Edge-MLP weights use an [o,i] (cout-major) column order so the per-edge
   h[src] multiply broadcasts over o with a packed-i last dim (DVE 2x mode).
 - ea is packed 3 chunks per 128-col group at partition bases {0,32,64}
   (matmul base-partition constraint) so the one-time DMA is ~8us, and layers
   1/3 compute z for 3 chunks in ONE matmul via a block-diagonal lw.
 - Per 128-edge chunk (layer 2): PE z matmul -> PSUM fp32; drain split between
   DVE (fused relu*h scalar_tensor_tensor on some cols) and ACT (relu) + DVE
   (2x bf16 multiply); PE mask-matmul scatter-accumulates into agg PSUM.
 - Root/bias injections go into spare agg columns [cc:cc+cout] (fp32 matmuls);
   one strided tensor_reduce folds the cin sum; scatter-mean via
   reciprocal-count multiply.
 - h is AllGathered between layers (bf16, rows padded to 256B); the 15us
   collective latency is hidden by emitting the next layer's (h-independent)
   z matmuls + ACT relus ahead with deep prelu buffering. h[src] gathers are
   slab-batched gpsimd dma_gather ops (int16 indices, 256B rows).
 - CBT: layer-3 h is AllGathered in fp32; each core computes its 64 output
   rows in fp32 (two column halves to bound SBUF).
"""
import os
import sys

for _p in ("/opt/trn_rl_repo", os.path.expanduser("~/.axon_site/_ro/trn_rl_repo")):
    if os.path.isdir(_p) and _p not in sys.path:
        sys.path.insert(0, _p)

import numpy as np

import concourse.bass as bass
import concourse.bacc as bacc
import concourse.tile as tile
from concourse import mybir
from concourse.bass import IndirectOffsetOnAxis
from concourse.bass_utils import run_bass_kernel_spmd

F32 = mybir.dt.float32
BF16 = mybir.dt.bfloat16
I32 = mybir.dt.int32
ALU = mybir.AluOpType
AXL = mybir.AxisListType
RELU = mybir.ActivationFunctionType.Relu
NPBF = mybir.dt.np(BF16)

V = 4
DIMS = [(1, 36), (36, 24), (24, 8)]
P = 128
SLAB = 16
NPRO2 = 24     # layer-2 prologue depth (chunks) hiding the AllGather
NCATCH2 = 36   # layer-2 chunks after prologue that stay in ACT-full mode
STT2 = 6       # layer-2 o-groups handled by fused DVE stt (of cout=24)
NPRO3 = 20     # layer-3 prologue depth (triples)
NCATCH3 = 32
STT3 = 2       # layer-3 o-groups (of cout=8) per chunk on DVE stt


def _ap(t, dims, pdim=None):
    p0 = list(t.ap[0]) if pdim is None else [pdim[0], pdim[1]]
    return bass.AP(tensor=t.tensor, offset=t.offset, ap=[p0] + [[s, c] for s, c in dims])


def _apo(t, off, dims):
    """AP into tile/AP `t` at free-element offset `off` with explicit free dims."""
    return bass.AP(tensor=t.tensor, offset=t.offset + off,
                   ap=[list(t.ap[0])] + [[s, c] for s, c in dims])


def _host_prep(x, edge_attr, edge_index, n_cores):
    src = np.asarray(edge_index[0]).astype(np.int64)
    dst = np.asarray(edge_index[1]).astype(np.int64)
    ea = np.asarray(edge_attr, dtype=np.float32)
    nn = int(np.asarray(x).shape[0])
    npc = nn // n_cores

    cnt = np.bincount(dst, minlength=nn).astype(np.float32)
    recip = (1.0 / np.maximum(cnt, 1.0)).astype(np.float32)

    perm = np.argsort(dst, kind="stable")
    src_s, dst_s = src[perm], dst[perm]
    ea_s = ea[perm]
    bounds = np.searchsorted(dst_s, np.arange(0, nn + 1, npc))
    n_chunks = int(np.ceil(np.diff(bounds).max() / P))
    n_chunks = max(3, 3 * int(np.ceil(n_chunks / 3)))  # pad to triple multiple
    e_pad = n_chunks * P
    cpb = n_chunks // 3

    cores = []
    for c in range(n_cores):
        lo, hi = int(bounds[c]), int(bounds[c + 1])
        k = hi - lo
        # ea69: chunk c -> partition base 32*(c%3), col group c//3
        ea5 = np.zeros((5, e_pad), dtype=np.float32)
        ea5[:4, :k] = ea_s[lo:hi].T
        ea5[4, :k] = 1.0
        ea69 = np.zeros((69, cpb * P), dtype=NPBF)
        for b in range(3):
            blk = ea5.reshape(5, n_chunks, P)[:, b::3, :].reshape(5, cpb * P)
            ea69[32 * b:32 * b + 5, :] = blk.astype(NPBF)
        srcc = np.zeros((e_pad,), dtype=np.int32)
        srcc[:k] = src_s[lo:hi].astype(np.int32)
        dloc = np.zeros((e_pad,), dtype=np.int64)
        dloc[:k] = dst_s[lo:hi] - c * npc
        ar = np.arange(e_pad)
        masks = np.zeros((n_chunks, P, npc), dtype=NPBF)
        masks[ar // P, ar % P, dloc] = (ar < k).astype(NPBF)
        ccnt = np.maximum(cnt[c * npc:(c + 1) * npc], 1.0).astype(np.float32)
        xloc = np.asarray(x, np.float32)[c * npc:(c + 1) * npc].reshape(npc)
        cores.append(
            dict(
                ea69=ea69,
                offs_t=np.ascontiguousarray(srcc.reshape(n_chunks, P).T),
                masks_t=np.ascontiguousarray(masks.transpose(1, 0, 2)),
                recip=recip[c * npc:(c + 1) * npc].reshape(-1, 1).copy(),
                cntrow=ccnt.reshape(1, npc).copy(),
                cntrep=np.ascontiguousarray(
                    np.broadcast_to(ccnt[None, :], (36, npc))).copy(),
                xcnt=(xloc * ccnt).reshape(1, npc).copy(),
            )
        )
    return cores, n_chunks


def _perm_oi(lw5, cin, cout):
    """[5, cin*cout] in (i,o) order -> (o,i) order."""
    return np.ascontiguousarray(
        lw5.reshape(5, cin, cout).transpose(0, 2, 1).reshape(5, cin * cout))


def _build_program(nn, n_cores, n_chunks):
    npc = nn // n_cores
    nc = bacc.Bacc()
    cpb = n_chunks // 3
    e_pad = n_chunks * P
    n_slabs = (n_chunks + SLAB - 1) // SLAB
    n_tri = n_chunks // 3

    ea_d = nc.declare_dram_parameter("ea69", [69, cpb * P], BF16, isOutput=False)
    offs_d = nc.declare_dram_parameter("offs_t", [P, n_chunks], I32, isOutput=False)
    masks_d = nc.declare_dram_parameter("masks_t", [P, n_chunks, npc], BF16, isOutput=False)
    recip_d = nc.declare_dram_parameter("recip", [npc, 1], F32, isOutput=False)
    cntrow_d = nc.declare_dram_parameter("cntrow", [1, npc], F32, isOutput=False)
    cntrep_d = nc.declare_dram_parameter("cntrep", [36, npc], F32, isOutput=False)
    xcnt_d = nc.declare_dram_parameter("xcnt", [1, npc], F32, isOutput=False)
    lwr_d, lwbd_d, root_d, bias_d = [], [], [], []
    for li, (cin, cout) in enumerate(DIMS):
        cc = cin * cout
        lwr_d.append(nc.declare_dram_parameter(f"lwr_{li}", [69, cc], BF16, isOutput=False))
        lwbd_d.append(nc.declare_dram_parameter(f"lwbd_{li}", [69, 3 * cc], BF16, isOutput=False))
        root_d.append(nc.declare_dram_parameter(f"root_{li}", [cin, cout], F32, isOutput=False))
        bias_d.append(nc.declare_dram_parameter(f"bias_{li}", [1, cout], F32, isOutput=False))
    out_d = nc.declare_dram_parameter("out_cbt", [npc, nn], F32, isOutput=True)
    dbg_d = None
    if os.environ.get("K_DEBUG_H"):
        dbg_d = [nc.declare_dram_parameter(f"dbg_h{li}", [nn, cout],
                                           BF16 if li < 2 else F32, isOutput=True)
                 for li, (_, cout) in enumerate(DIMS)]

    CC2 = DIMS[1][0] * DIMS[1][1]       # 864
    SPLITS2 = [(0, 512), (512, CC2 - 512)]

    with tile.TileContext(nc) as tc:
        with (
            tc.tile_pool(name="consts", bufs=1) as consts,
            tc.tile_pool(name="hs", bufs=3) as hs_pool,
            tc.tile_pool(name="pre2", bufs=NPRO2 + 4) as pre2_pool,
            tc.tile_pool(name="pre3", bufs=NPRO3 + 4) as pre3_pool,
            tc.tile_pool(name="pp", bufs=8) as p_pool,
            tc.tile_pool(name="sm", bufs=1) as sm_pool,
            tc.tile_pool(name="cb", bufs=1) as cb_pool,
            tc.tile_pool(name="zp", bufs=3, space="PSUM") as z_pool,
            tc.tile_pool(name="ag", bufs=1, space="PSUM") as ag_pool,
            tc.tile_pool(name="dr", bufs=1, space="DRAM") as dram,
        ):
            # ---- one-time constants ----
            lwr_s, lwbd_s, root_s, bias_s = [], [], [], []
            for li, (cin, cout) in enumerate(DIMS):
                cc = cin * cout
                if li == 1:
                    t = consts.tile([69, cc], BF16, name=f"lwr_{li}")
                    nc.sync.dma_start(out=t, in_=lwr_d[li][:, :])
                else:
                    t = None
                lwr_s.append(t)
                if li != 1:
                    t = consts.tile([69, 3 * cc], BF16, name=f"lwbd_{li}")
                    nc.sync.dma_start(out=t, in_=lwbd_d[li][:, :])
                else:
                    t = None
                lwbd_s.append(t)
                r = consts.tile([cin, cout], F32, name=f"root_{li}")
                nc.sync.dma_start(out=r, in_=root_d[li][:, :])
                root_s.append(r)
                b = consts.tile([1, cout], F32, name=f"bias_{li}")
                nc.sync.dma_start(out=b, in_=bias_d[li][:, :])
                bias_s.append(b)
            recip_s = consts.tile([npc, 1], F32)
            nc.sync.dma_start(out=recip_s, in_=recip_d[:, :])
            cntrow_s = consts.tile([1, npc], F32)
            nc.sync.dma_start(out=cntrow_s, in_=cntrow_d[:, :])
            cntrep_s = consts.tile([36, npc], F32)
            nc.sync.dma_start(out=cntrep_s, in_=cntrep_d[:, :])
            xcnt_s = consts.tile([1, npc], F32)
            nc.sync.dma_start(out=xcnt_s, in_=xcnt_d[:, :])
            offs_s = consts.tile([P, n_chunks], I32)
            nc.sync.dma_start(out=offs_s, in_=offs_d[:, :])

            # ea69: split into 8 column pieces; first on SP, rest on gpsimd
            ea_s = consts.tile([69, cpb * P], BF16, name="ea69")
            cols = cpb * P
            step = max(P, ((cols // 8) // P) * P)
            for pi, s0 in enumerate(range(0, cols, step)):
                s1 = min(cols, s0 + step)
                eng = nc.sync if pi == 0 else nc.gpsimd
                eng.dma_start(out=ea_s[:, s0:s1], in_=ea_d[:, s0:s1])

            mask_s = []
            for sl in range(n_slabs):
                c0 = sl * SLAB
                c1 = min(n_chunks, c0 + SLAB)
                t = consts.tile([P, c1 - c0, npc], BF16, name=f"mask_{sl}")
                nc.sync.dma_start(out=t, in_=masks_d[:, c0:c1, :])
                mask_s.append(t)

            # bf16 h (gather source, layers 0/1) + f32 h (inj, CBT)
            h_locp = [dram.tile([npc, DIMS[li][1]], BF16, name=f"hlocp_{li}") for li in range(2)]
            h_fullp = [dram.tile([nn, DIMS[li][1]], BF16, name=f"hfullp_{li}") for li in range(2)]
            h_locf = [dram.tile([npc, cout], F32, name=f"hlocf_{li}")
                      for li, (_, cout) in enumerate(DIMS)]
            h_f3 = dram.tile([nn, DIMS[2][1]], F32, name="hfull3f")

            agg = ag_pool.tile([npc, 928], F32, tag="agg", name="agg")

            def mask_ap(c):
                return mask_s[c // SLAB][:, c % SLAB, :]

            def epilogue(li, cin, cout, cc):
                if li == 0:
                    lhst = xcnt_s
                else:
                    h_t = sm_pool.tile([36, npc], F32, tag="ht", name=f"ht_{li}")
                    nc.sync.dma_start(
                        out=h_t[:cin, :],
                        in_=_ap(h_locf[li - 1][:, :], [(cin, npc)], pdim=(1, cin)),
                    )
                    h_tc = sm_pool.tile([36, npc], F32, tag="htc", name=f"htc_{li}")
                    nc.vector.tensor_tensor(
                        out=h_tc[:cin, :], in0=h_t[:cin, :], in1=cntrep_s[:cin, :],
                        op=ALU.mult)
                    lhst = h_tc[:cin, :]
                nc.tensor.matmul(agg[:, cc:cc + cout], lhsT=lhst, rhs=root_s[li],
                                 start=True, stop=False)
                nc.tensor.matmul(agg[:, cc:cc + cout], lhsT=cntrow_s, rhs=bias_s[li],
                                 start=False, stop=True)
                red = sm_pool.tile([npc, cout], F32, tag="red", name=f"red_{li}")
                nc.vector.tensor_reduce(
                    out=red, in_=_ap(agg, [(cin, cout), (1, cin)]),
                    axis=AXL.X, op=ALU.add)
                tot = sm_pool.tile([npc, cout], F32, tag="tot", name=f"tot_{li}")
                nc.vector.tensor_tensor(out=tot, in0=red, in1=agg[:, cc:cc + cout],
                                        op=ALU.add)
                h_s = sm_pool.tile([npc, cout], F32, tag="hsf", name=f"hsf_{li}")
                nc.vector.tensor_scalar(h_s, tot, recip_s[:, 0:1], 0.0, ALU.mult, ALU.max)
                nc.sync.dma_start(out=h_locf[li][:, :], in_=h_s)
                if li < 2:
                    h_bf = sm_pool.tile([npc, cout], BF16, tag="hbf", name=f"hbf_{li}")
                    nc.vector.tensor_scalar(h_bf, h_s, 1.0, None, ALU.mult)
                    nc.sync.dma_start(out=h_locp[li][:, :], in_=h_bf)
                    nc.gpsimd.collective_compute(
                        "AllGather", ALU.bypass,
                        replica_groups=[list(range(n_cores))],
                        ins=[h_locp[li].opt()], outs=[h_fullp[li].opt()])
                else:
                    nc.gpsimd.collective_compute(
                        "AllGather", ALU.bypass,
                        replica_groups=[list(range(n_cores))],
                        ins=[h_locf[li].opt()], outs=[h_f3.opt()])
                return h_s

            def gather_layer(li_prev, tag):
                cin_l = DIMS[li_prev][1]
                out = []
                for sl in range(n_slabs):
                    c0 = sl * SLAB
                    cs = min(n_chunks, c0 + SLAB) - c0
                    t = hs_pool.tile([P, SLAB, cin_l], BF16, tag=tag,
                                     name=f"{tag}_{sl}")
                    for si in range(cs):
                        nc.gpsimd.indirect_dma_start(
                            out=t[:, si, :], out_offset=None,
                            in_=h_fullp[li_prev][:, :],
                            in_offset=IndirectOffsetOnAxis(
                                ap=offs_s[:, c0 + si:c0 + si + 1], axis=0))
                    out.append(t)
                return out

            # ================= layer 0 (cin=1: msg = relu(z)) =================
            cin, cout = DIMS[0]
            cc = cin * cout
            for t in range(n_tri):
                zt = z_pool.tile([P, 864], F32, tag="z", name=f"z1_{t}")
                z3 = zt[:, 0:3 * cc]
                nc.tensor.matmul(z3, lhsT=ea_s[0:69, t * P:(t + 1) * P],
                                 rhs=lwbd_s[0], start=True, stop=True)
                p3 = p_pool.tile([P, 3 * cc], BF16, tag="p1", name=f"p1_{t}")
                if t % 2 == 0:
                    nc.vector.tensor_scalar(p3, z3, 0.0, None, ALU.max)
                else:
                    nc.scalar.activation(out=p3, in_=z3, func=RELU)
                for b in range(3):
                    c = 3 * t + b
                    nc.tensor.matmul(
                        agg[:, 0:cc], lhsT=mask_ap(c), rhs=p3[:, b * cc:(b + 1) * cc],
                        start=(c == 0), stop=(c == n_chunks - 1))
            epilogue(0, cin, cout, cc)

            # ================= layer 1 (the big one, cc=864) =================
            cin, cout = DIMS[1]
            cc = cin * cout
            hs2 = gather_layer(0, "hs2")

            def emit_z2(c):
                b, tg = c % 3, c // 3
                z = z_pool.tile([P, cc], F32, tag="z", name=f"z2_{c}")
                for off, n in SPLITS2:
                    nc.tensor.matmul(
                        z[:, off:off + n],
                        lhsT=ea_s[32 * b:32 * b + 5, tg * P:(tg + 1) * P],
                        rhs=lwr_s[1][32 * b:32 * b + 5, off:off + n],
                        start=True, stop=True)
                return z

            def emit_relu2(c, z, full):
                pre = pre2_pool.tile([P, cc], BF16, tag="pre2", name=f"pre2_{c}")
                o0 = 0 if full else STT2 * cin
                nc.scalar.activation(out=pre[:, o0:cc], in_=z[:, o0:cc], func=RELU)
                return pre

            zq, preq = {}, {}
            for c in range(min(NPRO2, n_chunks)):
                zq[c] = emit_z2(c)
                preq[c] = emit_relu2(c, zq[c], True)
            for c in range(n_chunks):
                full = c < NPRO2 + NCATCH2
                z, pre = zq.pop(c), preq.pop(c)
                hsv = hs2[c // SLAB][:, c % SLAB, :]
                p_t = p_pool.tile([P, cc], BF16, tag="p2", name=f"p2_{c}")
                if full:
                    nc.vector.tensor_tensor(
                        out=_ap(p_t, [(cin, cout), (1, cin)]),
                        in0=_ap(pre, [(cin, cout), (1, cin)]),
                        in1=_ap(hsv, [(0, cout), (1, cin)]), op=ALU.mult)
                else:
                    nc.vector.scalar_tensor_tensor(
                        out=_ap(p_t, [(cin, STT2), (1, cin)]),
                        in0=_ap(z, [(cin, STT2), (1, cin)]), scalar=0.0,
                        in1=_ap(hsv, [(0, STT2), (1, cin)]),
                        op0=ALU.max, op1=ALU.mult)
                    o0 = STT2 * cin
                    nc.vector.tensor_tensor(
                        out=_apo(p_t, o0, [(cin, cout - STT2), (1, cin)]),
                        in0=_apo(pre, o0, [(cin, cout - STT2), (1, cin)]),
                        in1=_ap(hsv, [(0, cout - STT2), (1, cin)]), op=ALU.mult)
                for off, n in SPLITS2:
                    nc.tensor.matmul(
                        agg[:, off:off + n], lhsT=mask_ap(c), rhs=p_t[:, off:off + n],
                        start=(c == 0), stop=(c == n_chunks - 1))
                if c + NPRO2 < n_chunks:
                    c2 = c + NPRO2
                    zq[c2] = emit_z2(c2)
                    preq[c2] = emit_relu2(c2, zq[c2], c2 < NPRO2 + NCATCH2)
            epilogue(1, cin, cout, cc)

            # ================= layer 2 (cc=192, triples) =================
            cin, cout = DIMS[2]
            cc = cin * cout
            hs3 = gather_layer(1, "hs3")

            def emit_z3(t):
                zt = z_pool.tile([P, 864], F32, tag="z", name=f"z3_{t}")
                z = zt[:, 0:3 * cc]
                for off, n in ((0, 512), (512, 3 * cc - 512)):
                    nc.tensor.matmul(z[:, off:off + n],
                                     lhsT=ea_s[0:69, t * P:(t + 1) * P],
                                     rhs=lwbd_s[2][:, off:off + n],
                                     start=True, stop=True)
                return z

            def emit_relu3(t, z, full):
                pre = pre3_pool.tile([P, 3 * cc], BF16, tag="pre3", name=f"pre3_{t}")
                if full:
                    nc.scalar.activation(out=pre, in_=z, func=RELU)
                else:
                    o0 = STT3 * cin
                    nc.scalar.activation(
                        out=_apo(pre, o0, [(cc, 3), (1, cc - o0)]),
                        in_=_apo(z, o0, [(cc, 3), (1, cc - o0)]),
                        func=RELU)
                return pre

            zq3, preq3 = {}, {}
            for t in range(min(NPRO3, n_tri)):
                zq3[t] = emit_z3(t)
                preq3[t] = emit_relu3(t, zq3[t], True)
            for t in range(n_tri):
                full = t < NPRO3 + NCATCH3
                z, pre = zq3.pop(t), preq3.pop(t)
                c0 = 3 * t
                p_t = p_pool.tile([P, 3 * cc], BF16, tag="p3", name=f"p3_{t}")
                for b in range(3):
                    c = c0 + b
                    hsv = hs3[c // SLAB][:, c % SLAB, :]
                    if full:
                        nc.vector.tensor_tensor(
                            out=_apo(p_t, b * cc, [(cin, cout), (1, cin)]),
                            in0=_apo(pre, b * cc, [(cin, cout), (1, cin)]),
                            in1=_ap(hsv, [(0, cout), (1, cin)]), op=ALU.mult)
                    else:
                        o0 = STT3 * cin
                        nc.vector.scalar_tensor_tensor(
                            out=_apo(p_t, b * cc, [(cin, STT3), (1, cin)]),
                            in0=_apo(z, b * cc, [(cin, STT3), (1, cin)]),
                            scalar=0.0, in1=_ap(hsv, [(0, STT3), (1, cin)]),
                            op0=ALU.max, op1=ALU.mult)
                        nc.vector.tensor_tensor(
                            out=_apo(p_t, b * cc + o0, [(cin, cout - STT3), (1, cin)]),
                            in0=_apo(pre, b * cc + o0, [(cin, cout - STT3), (1, cin)]),
                            in1=_ap(hsv, [(0, cout - STT3), (1, cin)]), op=ALU.mult)
                for b in range(3):
                    c = c0 + b
                    nc.tensor.matmul(
                        agg[:, 0:cc], lhsT=mask_ap(c), rhs=p_t[:, b * cc:(b + 1) * cc],
                        start=(c == 0), stop=(c == n_chunks - 1))
                if t + NPRO3 < n_tri:
                    t2 = t + NPRO3
                    zq3[t2] = emit_z3(t2)
                    preq3[t2] = emit_relu3(t2, zq3[t2], t2 < NPRO3 + NCATCH3)
            h3_s = epilogue(2, cin, cout, cc)
            if dbg_d is not None:
                nc.sync.dma_start(out=dbg_d[0][:, :], in_=h_fullp[0][:, :])
                nc.sync.dma_start(out=dbg_d[1][:, :], in_=h_fullp[1][:, :])
                nc.sync.dma_start(out=dbg_d[2][:, :], in_=h_f3[:, :])

            # ================= CBT (fp32, two column halves) =================
            dlast = DIMS[-1][1]
            half = nn // 2
            for hh in range(2):
                j0 = hh * half
                hrep = cb_pool.tile([npc, half, dlast], F32, tag="hrep",
                                    name=f"hrep_{hh}")
                nc.sync.dma_start(
                    out=hrep.rearrange("p j d -> p (j d)"),
                    in_=bass.AP(tensor=h_f3.tensor, offset=h_f3.offset + j0 * dlast,
                                ap=[[0, npc], [1, half * dlast]]))
                diff = cb_pool.tile([npc, half, dlast], F32, tag="diff",
                                    name=f"diff_{hh}")
                nc.vector.tensor_tensor(
                    out=diff[:], in0=hrep[:],
                    in1=_ap(h3_s[:], [(0, half), (1, dlast)]), op=ALU.subtract)
                cbt = cb_pool.tile([npc, half], F32, tag="cbt", name=f"cbt_{hh}")
                nc.vector.tensor_reduce(
                    out=cbt, in_=diff[:], axis=AXL.X, op=ALU.add,
                    apply_absolute_value=True)
                nc.sync.dma_start(out=out_d[:, j0:j0 + half], in_=cbt[:])
    return nc


def _make_in_maps(inputs, cores):
    x = np.asarray(inputs["x"], np.float32)
    x0 = float(x[0, 0])
    in_maps = []
    for c in range(len(cores)):
        m = dict(
            ea69=cores[c]["ea69"], offs_t=cores[c]["offs_t"],
            masks_t=cores[c]["masks_t"], recip=cores[c]["recip"],
            cntrow=cores[c]["cntrow"], cntrep=cores[c]["cntrep"],
            xcnt=cores[c]["xcnt"],
        )
        for li, (cin, cout) in enumerate(DIMS):
            cc = cin * cout
            lw = np.asarray(inputs[f"lin_w{li + 1}"], np.float32)
            lb = np.asarray(inputs[f"lin_b{li + 1}"], np.float32)
            lw5 = np.vstack([lw, lb[None, :]]).astype(np.float32)
            if li == 0:
                lw5 = lw5 * x0
            lw5 = _perm_oi(lw5, cin, cout).astype(NPBF)
            lwr = np.zeros((69, cc), dtype=NPBF)
            lwbd = np.zeros((69, 3 * cc), dtype=NPBF)
            for b in range(3):
                lwr[32 * b:32 * b + 5, :] = lw5
                lwbd[32 * b:32 * b + 5, b * cc:(b + 1) * cc] = lw5
            m[f"lwr_{li}"] = lwr
            m[f"lwbd_{li}"] = lwbd
            m[f"root_{li}"] = np.asarray(inputs[f"root{li + 1}"], np.float32)
            m[f"bias_{li}"] = np.asarray(
                inputs[f"bias{li + 1}"], np.float32).reshape(1, -1)
        in_maps.append(m)
    return in_maps


def _run(inputs, n_cores, sim=False):
    x = np.asarray(inputs["x"], np.float32)
    nn = x.shape[0]
    x0 = float(x[0, 0])
    assert np.all(x == x0) and x0 >= 0.0, "general-x path not implemented"
    cores, n_chunks = _host_prep(x, inputs["edge_attr"], inputs["edge_index"], n_cores)
    nc = _build_program(nn, n_cores, n_chunks)
    nc.finalize()
    in_maps = _make_in_maps(inputs, cores)

    global _LAST
    _LAST = (nc, in_maps)
    if sim:
        from concourse.bass_interp import MultiCoreSim

        ms = MultiCoreSim(nc, n_cores)
        for c in range(n_cores):
            for k, v in in_maps[c].items():
                ms.cores[c].tensor(k)[:] = v
        ms.simulate()
        rows = [np.asarray(ms.cores[c].tensor("out_cbt")) for c in range(n_cores)]
    else:
        res = run_bass_kernel_spmd(nc, in_maps, list(range(n_cores)))
        rows = [res.results[c]["out_cbt"] for c in range(n_cores)]
    return np.concatenate(rows, 0).astype(np.float32)


_LAST = None


def kernel(**inputs) -> np.ndarray:
    return _run(inputs, n_cores=8, sim=False)


# revision 23
# speedup vs baseline: 1.0870x; 1.0870x over previous
"""Trainium2 Bass kernel for DGN-style GNN message passing (3x NNConv + pairwise L1 CBT).

Strategy (8 NeuronCores, SPMD, edges sharded by destination node):
 - Core c owns nodes [64c, 64c+64) and all edges targeting them (host sorts
   edges by dst, pads per-core lists to a common chunk count; 128 edges/chunk).
 - Hot tensors are bf16 (PE matmuls are 4x faster than fp32; DVE gets 2x on
   packed bf16 SBUF ops). PSUM stays fp32. Quantization errors on the edge
   path average out over ~512 edges/node; the non-averaging paths (root
   injection, final CBT) run in fp32.
 - Edge-MLP weights use an [o,i] (cout-major) column order so the per-edge
   h[src] multiply broadcasts over o with a packed-i last dim (DVE 2x mode).
 - ea is packed 3 chunks per 128-col group at partition bases {0,32,64}
   (matmul base-partition constraint) so the one-time DMA is ~8us, and layers
   1/3 compute z for 3 chunks in ONE matmul via a block-diagonal lw.
 - Per 128-edge chunk (layer 2): PE z matmul -> PSUM fp32; drain split between
   DVE (fused relu*h scalar_tensor_tensor on some cols) and ACT (relu) + DVE
   (2x bf16 multiply); PE mask-matmul scatter-accumulates into agg PSUM.
 - Root/bias injections go into spare agg columns [cc:cc+cout] (fp32 matmuls);
   one strided tensor_reduce folds the cin sum; scatter-mean via
   reciprocal-count multiply.
 - h is AllGathered between layers (bf16, rows padded to 256B); the 15us
   collective latency is hidden by emitting the next layer's (h-independent)
   z matmuls + ACT relus ahead with deep prelu buffering. h[src] gathers are
   slab-batched gpsimd dma_gather ops (int16 indices, 256B rows).
 - CBT: layer-3 h is AllGathered in fp32; each core computes its 64 output
   rows in fp32 (two column halves to bound SBUF).
"""
import os
import sys

for _p in ("/opt/trn_rl_repo", os.path.expanduser("~/.axon_site/_ro/trn_rl_repo")):
    if os.path.isdir(_p) and _p not in sys.path:
        sys.path.insert(0, _p)

import numpy as np

import concourse.bass as bass
import concourse.bacc as bacc
import concourse.tile as tile
from concourse import mybir
from concourse.bass import IndirectOffsetOnAxis
from concourse.bass_utils import run_bass_kernel_spmd

F32 = mybir.dt.float32
BF16 = mybir.dt.bfloat16
I32 = mybir.dt.int32
ALU = mybir.AluOpType
AXL = mybir.AxisListType
RELU = mybir.ActivationFunctionType.Relu
NPBF = mybir.dt.np(BF16)

V = 4
DIMS = [(1, 36), (36, 24), (24, 8)]
P = 128
SLAB = 16
NPRO2 = 24     # layer-2 prologue depth (chunks) hiding the AllGather
NCATCH2 = 36   # layer-2 chunks after prologue that stay in ACT-full mode
STT2 = 6       # layer-2 o-groups handled by fused DVE stt (of cout=24)
NPRO3 = 20     # layer-3 prologue depth (triples)
NCATCH3 = 32
STT3 = 2       # layer-3 o-groups (of cout=8) per chunk on DVE stt


def _mode3(t):
    """Layer-3 gather engine for triple t: the 500ns/instr DMA floor is split
    across gpsimd-indirect (pool), and host-built one-hot DMAs on SP/ACT that
    feed a PE matmul gather. Ratios ~5:4:1 balance the three engine queues."""
    r = t % 10
    if r in (1, 3, 5, 7):
        return "pool"
    if r == 9:
        return "act"
    return "sp"


def _ap(t, dims, pdim=None):
    p0 = list(t.ap[0]) if pdim is None else [pdim[0], pdim[1]]
    return bass.AP(tensor=t.tensor, offset=t.offset, ap=[p0] + [[s, c] for s, c in dims])


def _apo(t, off, dims):
    """AP into tile/AP `t` at free-element offset `off` with explicit free dims."""
    return bass.AP(tensor=t.tensor, offset=t.offset + off,
                   ap=[list(t.ap[0])] + [[s, c] for s, c in dims])


def _host_prep(x, edge_attr, edge_index, n_cores):
    src = np.asarray(edge_index[0]).astype(np.int64)
    dst = np.asarray(edge_index[1]).astype(np.int64)
    ea = np.asarray(edge_attr, dtype=np.float32)
    nn = int(np.asarray(x).shape[0])
    npc = nn // n_cores

    cnt = np.bincount(dst, minlength=nn).astype(np.float32)
    recip = (1.0 / np.maximum(cnt, 1.0)).astype(np.float32)

    perm = np.argsort(dst, kind="stable")
    src_s, dst_s = src[perm], dst[perm]
    ea_s = ea[perm]
    bounds = np.searchsorted(dst_s, np.arange(0, nn + 1, npc))
    n_chunks = int(np.ceil(np.diff(bounds).max() / P))
    n_chunks = max(3, 3 * int(np.ceil(n_chunks / 3)))  # pad to triple multiple
    e_pad = n_chunks * P
    cpb = n_chunks // 3

    cores = []
    for c in range(n_cores):
        lo, hi = int(bounds[c]), int(bounds[c + 1])
        k = hi - lo
        # ea69: chunk c -> partition base 32*(c%3), col group c//3
        ea5 = np.zeros((5, e_pad), dtype=np.float32)
        ea5[:4, :k] = ea_s[lo:hi].T
        ea5[4, :k] = 1.0
        ea69 = np.zeros((69, cpb * P), dtype=NPBF)
        for b in range(3):
            blk = ea5.reshape(5, n_chunks, P)[:, b::3, :].reshape(5, cpb * P)
            ea69[32 * b:32 * b + 5, :] = blk.astype(NPBF)
        srcc = np.zeros((e_pad,), dtype=np.int32)
        srcc[:k] = src_s[lo:hi].astype(np.int32)
        dloc = np.zeros((e_pad,), dtype=np.int64)
        dloc[:k] = dst_s[lo:hi] - c * npc
        ar = np.arange(e_pad)
        masks = np.zeros((n_chunks, P, npc), dtype=NPBF)
        masks[ar // P, ar % P, dloc] = (ar < k).astype(NPBF)
        ccnt = np.maximum(cnt[c * npc:(c + 1) * npc], 1.0).astype(np.float32)
        xloc = np.asarray(x, np.float32)[c * npc:(c + 1) * npc].reshape(npc)
        n_tri = n_chunks // 3
        self_tris = [t for t in range(n_tri) if _mode3(t) != "pool"]
        oh3 = np.zeros((P, len(self_tris), 3, 4, P), dtype=NPBF)
        ee = np.arange(P)
        for j, t in enumerate(self_tris):
            for kk in range(3):
                sc = srcc[(3 * t + kk) * P:(3 * t + kk + 1) * P].astype(np.int64)
                oh3[sc % P, j, kk, sc // P, ee] = 1.0
        cores.append(
            dict(
                ea69=ea69,
                offs_t=np.ascontiguousarray(srcc.reshape(n_chunks, P).T),
                masks_t=np.ascontiguousarray(masks.transpose(1, 0, 2)),
                recip=recip[c * npc:(c + 1) * npc].reshape(-1, 1).copy(),
                cntrow=ccnt.reshape(1, npc).copy(),
                cntrep=np.ascontiguousarray(
                    np.broadcast_to(ccnt[None, :], (36, npc))).copy(),
                xcnt=(xloc * ccnt).reshape(1, npc).copy(),
                oh3=np.ascontiguousarray(oh3.reshape(P, -1)),
            )
        )
    return cores, n_chunks


def _perm_oi(lw5, cin, cout):
    """[5, cin*cout] in (i,o) order -> (o,i) order."""
    return np.ascontiguousarray(
        lw5.reshape(5, cin, cout).transpose(0, 2, 1).reshape(5, cin * cout))


def _build_program(nn, n_cores, n_chunks):
    npc = nn // n_cores
    nc = bacc.Bacc()
    cpb = n_chunks // 3
    e_pad = n_chunks * P
    n_slabs = (n_chunks + SLAB - 1) // SLAB
    n_tri = n_chunks // 3

    ea_d = nc.declare_dram_parameter("ea69", [69, cpb * P], BF16, isOutput=False)
    offs_d = nc.declare_dram_parameter("offs_t", [P, n_chunks], I32, isOutput=False)
    masks_d = nc.declare_dram_parameter("masks_t", [P, n_chunks, npc], BF16, isOutput=False)
    recip_d = nc.declare_dram_parameter("recip", [npc, 1], F32, isOutput=False)
    cntrow_d = nc.declare_dram_parameter("cntrow", [1, npc], F32, isOutput=False)
    cntrep_d = nc.declare_dram_parameter("cntrep", [36, npc], F32, isOutput=False)
    xcnt_d = nc.declare_dram_parameter("xcnt", [1, npc], F32, isOutput=False)
    lwr_d, lwbd_d, root_d, bias_d = [], [], [], []
    for li, (cin, cout) in enumerate(DIMS):
        cc = cin * cout
        lwr_d.append(nc.declare_dram_parameter(f"lwr_{li}", [69, cc], BF16, isOutput=False))
        lwbd_d.append(nc.declare_dram_parameter(f"lwbd_{li}", [69, 3 * cc], BF16, isOutput=False))
        root_d.append(nc.declare_dram_parameter(f"root_{li}", [cin, cout], F32, isOutput=False))
        bias_d.append(nc.declare_dram_parameter(f"bias_{li}", [1, cout], F32, isOutput=False))
    n_self3 = len([t for t in range(n_tri) if _mode3(t) != "pool"])
    oh3_d = nc.declare_dram_parameter("oh3", [P, n_self3 * 3 * 4 * P], BF16, isOutput=False)
    out_d = nc.declare_dram_parameter("out_cbt", [npc, nn], F32, isOutput=True)
    dbg_d = None
    if os.environ.get("K_DEBUG_H"):
        dbg_d = [nc.declare_dram_parameter(f"dbg_h{li}", [nn, cout],
                                           BF16 if li < 2 else F32, isOutput=True)
                 for li, (_, cout) in enumerate(DIMS)]

    CC2 = DIMS[1][0] * DIMS[1][1]       # 864
    SPLITS2 = [(0, 512), (512, CC2 - 512)]

    with tile.TileContext(nc) as tc:
        with (
            tc.tile_pool(name="consts", bufs=1) as consts,
            tc.tile_pool(name="hs", bufs=3) as hs_pool,
            tc.tile_pool(name="pre2", bufs=NPRO2 + 4) as pre2_pool,
            tc.tile_pool(name="pre3", bufs=NPRO3 + 4) as pre3_pool,
            tc.tile_pool(name="pp", bufs=8) as p_pool,
            tc.tile_pool(name="sm", bufs=1) as sm_pool,
            tc.tile_pool(name="cb", bufs=1) as cb_pool,
            tc.tile_pool(name="oh", bufs=4) as oh_pool,
            tc.tile_pool(name="zp", bufs=2, space="PSUM") as z_pool,
            tc.tile_pool(name="hp", bufs=2, space="PSUM") as hsp_pool,
            tc.tile_pool(name="ag", bufs=1, space="PSUM") as ag_pool,
            tc.tile_pool(name="dr", bufs=1, space="DRAM") as dram,
        ):
            # ---- one-time constants ----
            lwr_s, lwbd_s, root_s, bias_s = [], [], [], []
            for li, (cin, cout) in enumerate(DIMS):
                cc = cin * cout
                if li == 1:
                    t = consts.tile([69, cc], BF16, name=f"lwr_{li}")
                    nc.sync.dma_start(out=t, in_=lwr_d[li][:, :])
                else:
                    t = None
                lwr_s.append(t)
                if li != 1:
                    t = consts.tile([69, 3 * cc], BF16, name=f"lwbd_{li}")
                    nc.sync.dma_start(out=t, in_=lwbd_d[li][:, :])
                else:
                    t = None
                lwbd_s.append(t)
                r = consts.tile([cin, cout], F32, name=f"root_{li}")
                nc.sync.dma_start(out=r, in_=root_d[li][:, :])
                root_s.append(r)
                b = consts.tile([1, cout], F32, name=f"bias_{li}")
                nc.sync.dma_start(out=b, in_=bias_d[li][:, :])
                bias_s.append(b)
            recip_s = consts.tile([npc, 1], F32)
            nc.sync.dma_start(out=recip_s, in_=recip_d[:, :])
            cntrow_s = consts.tile([1, npc], F32)
            nc.sync.dma_start(out=cntrow_s, in_=cntrow_d[:, :])
            cntrep_s = consts.tile([36, npc], F32)
            nc.sync.dma_start(out=cntrep_s, in_=cntrep_d[:, :])
            xcnt_s = consts.tile([1, npc], F32)
            nc.sync.dma_start(out=xcnt_s, in_=xcnt_d[:, :])
            offs_s = consts.tile([P, n_chunks], I32)
            nc.sync.dma_start(out=offs_s, in_=offs_d[:, :])

            # ea69: split into 8 column pieces; first on SP, rest on gpsimd
            ea_s = consts.tile([69, cpb * P], BF16, name="ea69")
            cols = cpb * P
            step = max(P, ((cols // 8) // P) * P)
            for pi, s0 in enumerate(range(0, cols, step)):
                s1 = min(cols, s0 + step)
                eng = nc.sync if pi == 0 else nc.gpsimd
                eng.dma_start(out=ea_s[:, s0:s1], in_=ea_d[:, s0:s1])

            mask_s = []
            for sl in range(n_slabs):
                c0 = sl * SLAB
                c1 = min(n_chunks, c0 + SLAB)
                t = consts.tile([P, c1 - c0, npc], BF16, name=f"mask_{sl}")
                nc.sync.dma_start(out=t, in_=masks_d[:, c0:c1, :])
                mask_s.append(t)

            # bf16 h (gather source, layers 0/1) + f32 h (inj, CBT)
            h_locp = [dram.tile([npc, DIMS[li][1]], BF16, name=f"hlocp_{li}") for li in range(2)]
            h_fullp = [dram.tile([nn, DIMS[li][1]], BF16, name=f"hfullp_{li}") for li in range(2)]
            h_locf = [dram.tile([npc, cout], F32, name=f"hlocf_{li}")
                      for li, (_, cout) in enumerate(DIMS)]
            h_f3 = dram.tile([nn, DIMS[2][1]], F32, name="hfull3f")

            agg = ag_pool.tile([npc, 928], F32, tag="agg", name="agg")

            def mask_ap(c):
                return mask_s[c // SLAB][:, c % SLAB, :]

            def epilogue(li, cin, cout, cc):
                if li == 0:
                    lhst = xcnt_s
                else:
                    h_t = sm_pool.tile([36, npc], F32, tag="ht", name=f"ht_{li}")
                    nc.sync.dma_start(
                        out=h_t[:cin, :],
                        in_=_ap(h_locf[li - 1][:, :], [(cin, npc)], pdim=(1, cin)),
                    )
                    h_tc = sm_pool.tile([36, npc], F32, tag="htc", name=f"htc_{li}")
                    nc.vector.tensor_tensor(
                        out=h_tc[:cin, :], in0=h_t[:cin, :], in1=cntrep_s[:cin, :],
                        op=ALU.mult)
                    lhst = h_tc[:cin, :]
                nc.tensor.matmul(agg[:, cc:cc + cout], lhsT=lhst, rhs=root_s[li],
                                 start=True, stop=False)
                nc.tensor.matmul(agg[:, cc:cc + cout], lhsT=cntrow_s, rhs=bias_s[li],
                                 start=False, stop=True)
                red = sm_pool.tile([npc, cout], F32, tag="red", name=f"red_{li}")
                nc.vector.tensor_reduce(
                    out=red, in_=_ap(agg, [(cin, cout), (1, cin)]),
                    axis=AXL.X, op=ALU.add)
                tot = sm_pool.tile([npc, cout], F32, tag="tot", name=f"tot_{li}")
                nc.vector.tensor_tensor(out=tot, in0=red, in1=agg[:, cc:cc + cout],
                                        op=ALU.add)
                h_s = sm_pool.tile([npc, cout], F32, tag="hsf", name=f"hsf_{li}")
                nc.vector.tensor_scalar(h_s, tot, recip_s[:, 0:1], 0.0, ALU.mult, ALU.max)
                nc.sync.dma_start(out=h_locf[li][:, :], in_=h_s)
                if li < 2:
                    h_bf = sm_pool.tile([npc, cout], BF16, tag="hbf", name=f"hbf_{li}")
                    nc.vector.tensor_scalar(h_bf, h_s, 1.0, None, ALU.mult)
                    nc.sync.dma_start(out=h_locp[li][:, :], in_=h_bf)
                    nc.gpsimd.collective_compute(
                        "AllGather", ALU.bypass,
                        replica_groups=[list(range(n_cores))],
                        ins=[h_locp[li].opt()], outs=[h_fullp[li].opt()])
                else:
                    nc.gpsimd.collective_compute(
                        "AllGather", ALU.bypass,
                        replica_groups=[list(range(n_cores))],
                        ins=[h_locf[li].opt()], outs=[h_f3.opt()])
                return h_s

            def gather_layer(li_prev, tag, pool_only_modes=False):
                cin_l = DIMS[li_prev][1]
                out = []
                for sl in range(n_slabs):
                    c0 = sl * SLAB
                    cs = min(n_chunks, c0 + SLAB) - c0
                    t = hs_pool.tile([P, SLAB, cin_l], BF16, tag=tag,
                                     name=f"{tag}_{sl}")
                    for si in range(cs):
                        if pool_only_modes and _mode3((c0 + si) // 3) != "pool":
                            continue
                        nc.gpsimd.indirect_dma_start(
                            out=t[:, si, :], out_offset=None,
                            in_=h_fullp[li_prev][:, :],
                            in_offset=IndirectOffsetOnAxis(
                                ap=offs_s[:, c0 + si:c0 + si + 1], axis=0))
                    out.append(t)
                return out

            # ================= layer 0 (cin=1: msg = relu(z)) =================
            cin, cout = DIMS[0]
            cc = cin * cout
            for t in range(n_tri):
                zt = z_pool.tile([P, 864], F32, tag="z", name=f"z1_{t}")
                z3 = zt[:, 0:3 * cc]
                nc.tensor.matmul(z3, lhsT=ea_s[0:69, t * P:(t + 1) * P],
                                 rhs=lwbd_s[0], start=True, stop=True)
                p3 = p_pool.tile([P, 3 * cc], BF16, tag="p1", name=f"p1_{t}")
                if t % 2 == 0:
                    nc.vector.tensor_scalar(p3, z3, 0.0, None, ALU.max)
                else:
                    nc.scalar.activation(out=p3, in_=z3, func=RELU)
                for b in range(3):
                    c = 3 * t + b
                    nc.tensor.matmul(
                        agg[:, 0:cc], lhsT=mask_ap(c), rhs=p3[:, b * cc:(b + 1) * cc],
                        start=(c == 0), stop=(c == n_chunks - 1))
            epilogue(0, cin, cout, cc)

            # ================= layer 1 (the big one, cc=864) =================
            cin, cout = DIMS[1]
            cc = cin * cout
            hs2 = gather_layer(0, "hs2")

            def emit_z2(c):
                b, tg = c % 3, c // 3
                z = z_pool.tile([P, cc], F32, tag="z", name=f"z2_{c}")
                for off, n in SPLITS2:
                    nc.tensor.matmul(
                        z[:, off:off + n],
                        lhsT=ea_s[32 * b:32 * b + 5, tg * P:(tg + 1) * P],
                        rhs=lwr_s[1][32 * b:32 * b + 5, off:off + n],
                        start=True, stop=True)
                return z

            def emit_relu2(c, z, full):
                pre = pre2_pool.tile([P, cc], BF16, tag="pre2", name=f"pre2_{c}")
                o0 = 0 if full else STT2 * cin
                nc.scalar.activation(out=pre[:, o0:cc], in_=z[:, o0:cc], func=RELU)
                return pre

            zq, preq = {}, {}
            for c in range(min(NPRO2, n_chunks)):
                zq[c] = emit_z2(c)
                preq[c] = emit_relu2(c, zq[c], True)
            for c in range(n_chunks):
                full = c < NPRO2 + NCATCH2
                z, pre = zq.pop(c), preq.pop(c)
                hsv = hs2[c // SLAB][:, c % SLAB, :]
                p_t = p_pool.tile([P, cc], BF16, tag="p2", name=f"p2_{c}")
                if full:
                    nc.vector.tensor_tensor(
                        out=_ap(p_t, [(cin, cout), (1, cin)]),
                        in0=_ap(pre, [(cin, cout), (1, cin)]),
                        in1=_ap(hsv, [(0, cout), (1, cin)]), op=ALU.mult)
                else:
                    nc.vector.scalar_tensor_tensor(
                        out=_ap(p_t, [(cin, STT2), (1, cin)]),
                        in0=_ap(z, [(cin, STT2), (1, cin)]), scalar=0.0,
                        in1=_ap(hsv, [(0, STT2), (1, cin)]),
                        op0=ALU.max, op1=ALU.mult)
                    o0 = STT2 * cin
                    nc.vector.tensor_tensor(
                        out=_apo(p_t, o0, [(cin, cout - STT2), (1, cin)]),
                        in0=_apo(pre, o0, [(cin, cout - STT2), (1, cin)]),
                        in1=_ap(hsv, [(0, cout - STT2), (1, cin)]), op=ALU.mult)
                for off, n in SPLITS2:
                    nc.tensor.matmul(
                        agg[:, off:off + n], lhsT=mask_ap(c), rhs=p_t[:, off:off + n],
                        start=(c == 0), stop=(c == n_chunks - 1))
                if c + NPRO2 < n_chunks:
                    c2 = c + NPRO2
                    zq[c2] = emit_z2(c2)
                    preq[c2] = emit_relu2(c2, zq[c2], c2 < NPRO2 + NCATCH2)
            epilogue(1, cin, cout, cc)

            # ================= layer 2 (cc=192, triples) =================
            cin, cout = DIMS[2]
            cc = cin * cout
            hs3 = gather_layer(1, "hs3", pool_only_modes=True)
            # h2 resident in SBUF for the PE one-hot gather: [p, g, :] = h2[g*128+p]
            h2s = sm_pool.tile([P, 4, DIMS[1][1]], BF16, tag="h2s")
            nc.sync.dma_start(
                out=h2s.rearrange("p g d -> p (g d)"),
                in_=_ap(h_fullp[1][:, :],
                        [(P * DIMS[1][1], 4), (1, DIMS[1][1])],
                        pdim=(DIMS[1][1], P)))
            self_idx = {}
            for t in range(n_tri):
                if _mode3(t) != "pool":
                    self_idx[t] = len(self_idx)

            def emit_oh_dma(t):
                oh = oh_pool.tile([P, 3 * 4 * P], BF16, tag="oh", name=f"oh_{t}")
                j = self_idx[t] * 3 * 4 * P
                eng = nc.scalar if _mode3(t) == "act" else nc.sync
                eng.dma_start(out=oh, in_=oh3_d[:, j:j + 3 * 4 * P])
                return oh

            def emit_z3(t):
                zt = z_pool.tile([P, 864], F32, tag="z", name=f"z3_{t}")
                z = zt[:, 0:3 * cc]
                for off, n in ((0, 512), (512, 3 * cc - 512)):
                    nc.tensor.matmul(z[:, off:off + n],
                                     lhsT=ea_s[0:69, t * P:(t + 1) * P],
                                     rhs=lwbd_s[2][:, off:off + n],
                                     start=True, stop=True)
                return z

            def emit_relu3(t, z, full):
                pre = pre3_pool.tile([P, 3 * cc], BF16, tag="pre3", name=f"pre3_{t}")
                if full:
                    nc.scalar.activation(out=pre, in_=z, func=RELU)
                else:
                    o0 = STT3 * cin
                    nc.scalar.activation(
                        out=_apo(pre, o0, [(cc, 3), (1, cc - o0)]),
                        in_=_apo(z, o0, [(cc, 3), (1, cc - o0)]),
                        func=RELU)
                return pre

            zq3, preq3, ohq3 = {}, {}, {}
            for t in range(min(NPRO3, n_tri)):
                zq3[t] = emit_z3(t)
                preq3[t] = emit_relu3(t, zq3[t], True)
            for t in range(min(3, n_tri)):
                if t in self_idx:
                    ohq3[t] = emit_oh_dma(t)
            cin2 = DIMS[1][1]
            for t in range(n_tri):
                full = t < NPRO3 + NCATCH3
                z, pre = zq3.pop(t), preq3.pop(t)
                c0 = 3 * t
                if t in self_idx:
                    # PE one-hot gather: hsP[e, k*24+i] = h2[src[3t+k][e], i]
                    oh = ohq3.pop(t)
                    ohv = oh.rearrange("p (k g e) -> p k g e", k=3, g=4)
                    hsP = hsp_pool.tile([P, 3, cin], F32, tag="hsp", name=f"hsp_{t}")
                    for kk in range(3):
                        for g in range(4):
                            nc.tensor.matmul(
                                hsP[:, kk, :], lhsT=ohv[:, kk, g, :],
                                rhs=h2s[:, g, :cin],
                                start=(g == 0), stop=(g == 3))
                    hsl = p_pool.tile([P, 3, cin], BF16, tag="hsl", name=f"hsl_{t}")
                    nc.vector.tensor_scalar(
                        hsl.rearrange("p a b -> p (a b)"),
                        hsP.rearrange("p a b -> p (a b)"), 0.0, None, ALU.add)
                    hbase, hstep = hsl[:, 0, :], cin
                else:
                    sl, si = c0 // SLAB, c0 % SLAB
                    if si <= SLAB - 3:
                        hbase, hstep = hs3[sl][:, si, :], cin
                    else:
                        hbase, hstep = None, None
                p_t = p_pool.tile([P, 3 * cc], BF16, tag="p3", name=f"p3_{t}")
                o0 = STT3 * cin
                if hbase is not None:
                    if full:
                        nc.vector.tensor_tensor(
                            out=_ap(p_t, [(cc, 3), (cin, cout), (1, cin)]),
                            in0=_ap(pre, [(cc, 3), (cin, cout), (1, cin)]),
                            in1=_ap(hbase, [(hstep, 3), (0, cout), (1, cin)]),
                            op=ALU.mult)
                    else:
                        nc.vector.scalar_tensor_tensor(
                            out=_ap(p_t, [(cc, 3), (cin, STT3), (1, cin)]),
                            in0=_ap(z, [(cc, 3), (cin, STT3), (1, cin)]), scalar=0.0,
                            in1=_ap(hbase, [(hstep, 3), (0, STT3), (1, cin)]),
                            op0=ALU.max, op1=ALU.mult)
                        nc.vector.tensor_tensor(
                            out=_apo(p_t, o0, [(cc, 3), (cin, cout - STT3), (1, cin)]),
                            in0=_apo(pre, o0, [(cc, 3), (cin, cout - STT3), (1, cin)]),
                            in1=_ap(hbase, [(hstep, 3), (0, cout - STT3), (1, cin)]),
                            op=ALU.mult)
                else:
                    for b in range(3):
                        c = c0 + b
                        hsv = hs3[c // SLAB][:, c % SLAB, :]
                        if full:
                            nc.vector.tensor_tensor(
                                out=_apo(p_t, b * cc, [(cin, cout), (1, cin)]),
                                in0=_apo(pre, b * cc, [(cin, cout), (1, cin)]),
                                in1=_ap(hsv, [(0, cout), (1, cin)]), op=ALU.mult)
                        else:
                            nc.vector.scalar_tensor_tensor(
                                out=_apo(p_t, b * cc, [(cin, STT3), (1, cin)]),
                                in0=_apo(z, b * cc, [(cin, STT3), (1, cin)]),
                                scalar=0.0, in1=_ap(hsv, [(0, STT3), (1, cin)]),
                                op0=ALU.max, op1=ALU.mult)
                            nc.vector.tensor_tensor(
                                out=_apo(p_t, b * cc + o0,
                                         [(cin, cout - STT3), (1, cin)]),
                                in0=_apo(pre, b * cc + o0,
                                         [(cin, cout - STT3), (1, cin)]),
                                in1=_ap(hsv, [(0, cout - STT3), (1, cin)]),
                                op=ALU.mult)
                for b in range(3):
                    c = c0 + b
                    nc.tensor.matmul(
                        agg[:, 0:cc], lhsT=mask_ap(c), rhs=p_t[:, b * cc:(b + 1) * cc],
                        start=(c == 0), stop=(c == n_chunks - 1))
                if t + NPRO3 < n_tri:
                    t2 = t + NPRO3
                    zq3[t2] = emit_z3(t2)
                    preq3[t2] = emit_relu3(t2, zq3[t2], t2 < NPRO3 + NCATCH3)
                if t + 3 < n_tri and (t + 3) in self_idx:
                    ohq3[t + 3] = emit_oh_dma(t + 3)
            h3_s = epilogue(2, cin, cout, cc)
            if dbg_d is not None:
                nc.sync.dma_start(out=dbg_d[0][:, :], in_=h_fullp[0][:, :])
                nc.sync.dma_start(out=dbg_d[1][:, :], in_=h_fullp[1][:, :])
                nc.sync.dma_start(out=dbg_d[2][:, :], in_=h_f3[:, :])

            # ================= CBT (fp32, four column quarters) =================
            dlast = DIMS[-1][1]
            half = nn // 4
            for hh in range(4):
                j0 = hh * half
                hrep = cb_pool.tile([npc, half, dlast], F32, tag="hrep",
                                    name=f"hrep_{hh}")
                nc.sync.dma_start(
                    out=hrep.rearrange("p j d -> p (j d)"),
                    in_=bass.AP(tensor=h_f3.tensor, offset=h_f3.offset + j0 * dlast,
                                ap=[[0, npc], [1, half * dlast]]))
                diff = cb_pool.tile([npc, half, dlast], F32, tag="diff",
                                    name=f"diff_{hh}")
                nc.vector.tensor_tensor(
                    out=diff[:], in0=hrep[:],
                    in1=_ap(h3_s[:], [(0, half), (1, dlast)]), op=ALU.subtract)
                cbt = cb_pool.tile([npc, half], F32, tag="cbt", name=f"cbt_{hh}")
                nc.vector.tensor_reduce(
                    out=cbt, in_=diff[:], axis=AXL.X, op=ALU.add,
                    apply_absolute_value=True)
                nc.sync.dma_start(out=out_d[:, j0:j0 + half], in_=cbt[:])
    return nc


def _make_in_maps(inputs, cores):
    x = np.asarray(inputs["x"], np.float32)
    x0 = float(x[0, 0])
    in_maps = []
    for c in range(len(cores)):
        m = dict(
            ea69=cores[c]["ea69"], offs_t=cores[c]["offs_t"],
            masks_t=cores[c]["masks_t"], recip=cores[c]["recip"],
            cntrow=cores[c]["cntrow"], cntrep=cores[c]["cntrep"],
            xcnt=cores[c]["xcnt"], oh3=cores[c]["oh3"],
        )
        for li, (cin, cout) in enumerate(DIMS):
            cc = cin * cout
            lw = np.asarray(inputs[f"lin_w{li + 1}"], np.float32)
            lb = np.asarray(inputs[f"lin_b{li + 1}"], np.float32)
            lw5 = np.vstack([lw, lb[None, :]]).astype(np.float32)
            if li == 0:
                lw5 = lw5 * x0
            lw5 = _perm_oi(lw5, cin, cout).astype(NPBF)
            lwr = np.zeros((69, cc), dtype=NPBF)
            lwbd = np.zeros((69, 3 * cc), dtype=NPBF)
            for b in range(3):
                lwr[32 * b:32 * b + 5, :] = lw5
                lwbd[32 * b:32 * b + 5, b * cc:(b + 1) * cc] = lw5
            m[f"lwr_{li}"] = lwr
            m[f"lwbd_{li}"] = lwbd
            m[f"root_{li}"] = np.asarray(inputs[f"root{li + 1}"], np.float32)
            m[f"bias_{li}"] = np.asarray(
                inputs[f"bias{li + 1}"], np.float32).reshape(1, -1)
        in_maps.append(m)
    return in_maps


def _run(inputs, n_cores, sim=False):
    x = np.asarray(inputs["x"], np.float32)
    nn = x.shape[0]
    x0 = float(x[0, 0])
    assert np.all(x == x0) and x0 >= 0.0, "general-x path not implemented"
    cores, n_chunks = _host_prep(x, inputs["edge_attr"], inputs["edge_index"], n_cores)
    nc = _build_program(nn, n_cores, n_chunks)
    nc.finalize()
    in_maps = _make_in_maps(inputs, cores)

    global _LAST
    _LAST = (nc, in_maps)
    if sim:
        from concourse.bass_interp import MultiCoreSim

        ms = MultiCoreSim(nc, n_cores)
        for c in range(n_cores):
            for k, v in in_maps[c].items():
                ms.cores[c].tensor(k)[:] = v
        ms.simulate()
        rows = [np.asarray(ms.cores[c].tensor("out_cbt")) for c in range(n_cores)]
    else:
        res = run_bass_kernel_spmd(nc, in_maps, list(range(n_cores)))
        rows = [res.results[c]["out_cbt"] for c in range(n_cores)]
    return np.concatenate(rows, 0).astype(np.float32)


_LAST = None


def kernel(**inputs) -> np.ndarray:
    return _run(inputs, n_cores=8, sim=False)
